# revision 1
# baseline (speedup 1.0000x reference)
"""Depthwise 3D transposed conv (stride 2, k=4, SAME) on 8 trn2 NeuronCores.

x: (4, 32, 32, 32, 256) f32, filters: (4, 4, 4, 1, 256) f32
y: (4, 64, 64, 64, 256) f32

Sharding: 8 cores = (batch n in 4) x (d-halves in 2). Zero communication.

Math: polyphase decomposition of the stride-2 transposed conv. Per dim,
output parity p uses taps (delta, k):
  p=0: y[2m]   = f[3] x[m-1] + f[1] x[m]
  p=1: y[2m+1] = f[2] x[m]   + f[0] x[m+1]
Each output element is a sum of exactly 8 taps (2 per dim).

Compute (all TensorE, float32r = 1 cycle/row):
- Contraction folds the 2 d-taps: input tile partitions hold a PLANE PAIR,
  p = j*64 + cc <- (plane k+j, channel block cc of 64).
- Weight columns fold 2 OUTPUT PLANES: both output planes l=2k-1 (d-parity
  1) and l=2k (d-parity 0) read the same plane pair (k, k+1) with the same
  (dh, dw) window shifts, so a [128, 128] weight matrix with columns
  (r*64 + c'), W[(j,cc),(r,c')] = delta(cc,c') * F[kd(j, parity(r)), kh,
  kw, c'], computes partial sums for two planes in one matmul.
Each matmul thus covers 4 of the 8 taps for 2x64 channels x 512 positions
(256 useful MACs/cycle); 4 (dh,dw) taps accumulate per PSUM bank. ScalarE
evacuates PSUM->SBUF. ~2176 matmuls/core total.

Host pre-pairs planes (xp[k] = planes (k, k+1)) so every tile load is one
full-width 128-partition DMA with 4KB-contiguous per-partition runs. The
output accumulates in (plane-pair, polyphase)-major slabs stored as
contiguous [128, 16KB] DMAs; the host un-interleaves (and drops the two
out-of-range boundary plane slots).
"""
import sys

sys.path.insert(0, "/opt/trn_rl_repo")

from contextlib import ExitStack

import numpy as np

import concourse.bass as bass  # noqa: F401  (registers engine classes)
import concourse.tile as tile
from concourse import bacc, mybir
from concourse.bass_utils import run_bass_kernel_spmd

F32 = mybir.dt.float32
F32R = mybir.dt.float32r
F16 = mybir.dt.float16
AOP = mybir.AluOpType

N_CORES = 8
# per-dim taps: parity -> [(delta, k), ...]
TAPS = {0: [(-1, 3), (0, 1)], 1: [(0, 2), (1, 0)]}
PPS = ((0, 0), (0, 1), (1, 0), (1, 1))
NK = 17  # plane-pair tiles per core: k=0..16 holds local planes (k, k+1)

_PROG = None


def _widx(cg, s, ph, pw, t):
    """Flat index of the [128, 128] weight matrix for (cgroup, 64-ch strip,
    h/w polyphase, (dh, dw) tap index t in 0..3)."""
    return ((cg * 2 + s) * 4 + (ph * 2 + pw)) * 4 + t


def _build_program():
    nc = bacc.Bacc(
        "TRN2", target_bir_lowering=False, debug=False, num_devices=N_CORES
    )
    # xp: plane pairs, partition-ready: [k, q=(cg,s), j, cc, h, w]
    xp_d = nc.declare_dram_parameter("xp", [NK, 4, 2, 64, 34, 34], F16, isOutput=False)
    wd_d = nc.declare_dram_parameter("wpair", [128, 64, 128], F16, isOutput=False)
    # y: [k, cg, s, r, c', ph, pw, a, b]; plane l = 2k-1+r (r0 of k=0 and
    # r1 of k=16 are dropped by the host)
    y_d = nc.declare_dram_parameter(
        "y", [NK, 2, 2, 2, 64, 2, 2, 32, 32], F32, isOutput=True
    )

    with ExitStack() as ctx:
        tc = ctx.enter_context(tile.TileContext(nc))
        wpool = ctx.enter_context(tc.tile_pool(name="wpool", bufs=1))
        xpool = ctx.enter_context(tc.tile_pool(name="xpool", bufs=14))
        opool = ctx.enter_context(tc.tile_pool(name="opool", bufs=6))
        ppool = ctx.enter_context(tc.tile_pool(name="ppool", bufs=7, space="PSUM"))

        wd = wpool.tile([128, 64, 128], F16)
        wd_loaded = set()

        def load_wchunk(q):
            # lazy per-(cg,s) weight chunk: the first matmul only waits on
            # its own 16 matrices, not the whole table
            if q not in wd_loaded:
                nc.sync.dma_start(
                    out=wd[:, q * 16 : (q + 1) * 16, :],
                    in_=wd_d[:, q * 16 : (q + 1) * 16, :],
                )
                wd_loaded.add(q)

        def load_pair(k, cg, s):
            load_wchunk(cg * 2 + s)
            # halo border is pre-padded in DRAM: whole-tile contiguous load
            t = xpool.tile([128, 34, 34], F16, tag="xp")
            nc.sync.dma_start(
                out=t[:],
                in_=xp_d[k, cg * 2 + s].rearrange("j c h w -> (j c) h w"),
            )
            return t

        for k in range(NK):
            for cg in range(2):
                for s in range(2):
                    xt = load_pair(k, cg, s)
                    # out slab for 2 planes x 64 ch: [(r,c'), ph, pw, a, b]
                    ot = opool.tile([128, 2, 2, 32, 32], F32, tag="out")
                    for ph, pw in PPS:
                        hw_taps = [
                            (dh, kh, dw, kw)
                            for (dh, kh) in TAPS[ph]
                            for (dw, kw) in TAPS[pw]
                        ]
                        # taps outer, a-half inner: consecutive matmul pairs
                        # share a weight matrix (walrus ldw-opt dedups)
                        pss = [
                            ppool.tile([128, 16, 32], F32, tag="ps", name="ps")
                            for _ in range(2)
                        ]
                        for t_i, (dh, kh, dw, kw) in enumerate(hw_taps):
                            wap = wd[:, _widx(cg, s, ph, pw, t_i), :]
                            for ah in range(2):
                                a0 = ah * 16
                                win = xt[
                                    :,
                                    1 + a0 + dh : 1 + a0 + dh + 16,
                                    1 + dw : 1 + dw + 32,
                                ]
                                nc.tensor.matmul(
                                    pss[ah],
                                    wap,
                                    win,
                                    start=(t_i == 0),
                                    stop=(t_i == len(hw_taps) - 1),
                                )
                        for ah in range(2):
                            nc.scalar.copy(
                                ot[:, ph, pw, ah * 16 : ah * 16 + 16, :], pss[ah]
                            )
                    # SWDGE: keeps stores off the Sync FIFO so a
                    # blocked store never delays upcoming loads. Boundary
                    # pairs store only their valid plane half. Stores go
                    # out in two polyphase halves so the first half ships
                    # while the second half is still being evacuated.
                    for phh in range(2):
                        if k == 0:
                            nc.gpsimd.dma_start(
                                out=y_d[k, cg, s, 1, :, phh],
                                in_=ot[64:128, phh],
                            )
                        elif k == NK - 1:
                            nc.gpsimd.dma_start(
                                out=y_d[k, cg, s, 0, :, phh],
                                in_=ot[0:64, phh],
                            )
                        else:
                            nc.gpsimd.dma_start(
                                out=y_d[k, cg, s, :, :, phh], in_=ot[:, phh]
                            )
    nc.compile()
    return nc


def _get_program():
    global _PROG
    if _PROG is None:
        _PROG = _build_program()
    return _PROG


def _make_in_maps(x, filters):
    x = np.ascontiguousarray(np.asarray(x), dtype=np.float32)
    filters = np.asarray(filters, dtype=np.float32)
    ftap = filters[:, :, :, 0, :]  # (kd, kh, kw, c)

    # wpair[(j,cc), widx, (r,c')] = F[kd(j, parity(r)), kh, kw, cbase+c']
    #   * delta(cc, c');  r=0 -> parity 1 (l=2k-1), r=1 -> parity 0 (l=2k)
    wpair = np.zeros((128, 64, 128), np.float16)
    idx = np.arange(64)
    for cg in range(2):
        for s in range(2):
            cbase = cg * 128 + s * 64
            for ph, pw in PPS:
                taps = [(a, b) for a in TAPS[ph] for b in TAPS[pw]]
                for t, ((dh, kh), (dw, kw)) in enumerate(taps):
                    w = _widx(cg, s, ph, pw, t)
                    for r, pdr in ((0, 1), (1, 0)):
                        for j in range(2):
                            kd = TAPS[pdr][j][1]
                            wpair[j * 64 + idx, w, r * 64 + idx] = ftap[
                                kd, kh, kw, cbase : cbase + 64
                            ]

    in_maps = []
    for core in range(N_CORES):
        n, h = core // 2, core % 2
        lo = 16 * h - 1
        planes = np.zeros((18, 32, 32, 256), np.float32)
        s0, s1 = max(lo, 0), min(16 * h + 17, 32)
        planes[s0 - lo : s1 - lo] = x[n, s0:s1]
        planes = planes.transpose(0, 3, 1, 2)  # (18, 256, 32, 32)
        # pair planes with zero halo: xp[k, q, j, cc, 1+h, 1+w] =
        # planes[k+j, q*64+cc, h, w]
        pair = np.stack([planes[0:NK], planes[1 : NK + 1]], axis=1)
        pair = pair.reshape(NK, 2, 4, 64, 32, 32).transpose(0, 2, 1, 3, 4, 5)
        padded = np.zeros((NK, 4, 2, 64, 34, 34), np.float16)
        padded[:, :, :, :, 1:33, 1:33] = pair
        in_maps.append({"xp": padded, "wpair": wpair})
    return in_maps


def kernel(x, filters):
    nc = _get_program()
    in_maps = _make_in_maps(x, filters)
    res = run_bass_kernel_spmd(nc, in_maps, list(range(N_CORES)))
    y = np.empty((4, 64, 64, 64, 256), np.float32)
    for core in range(N_CORES):
        n, h = core // 2, core % 2
        yc = res.results[core]["y"]  # (k, cg, s, r, c', p, q, a, b)
        # l = 2k-1+r; ho = 2a+p; wo = 2b+q; c = cg*128 + s*64 + c'
        yc = yc.transpose(0, 3, 7, 5, 8, 6, 1, 2, 4)  # (k,r,a,p,b,q,cg,s,c')
        yc = yc.reshape(2 * NK, 64, 64, 256)[1 : 2 * NK - 1]
        y[n, 32 * h : 32 * h + 32] = yc
    return y



# revision 2
# speedup vs baseline: 1.0702x; 1.0702x over previous
"""Depthwise 3D transposed conv (stride 2, k=4, SAME) on 8 trn2 NeuronCores.

x: (4, 32, 32, 32, 256) f32, filters: (4, 4, 4, 1, 256) f32
y: (4, 64, 64, 64, 256) f32

Sharding: 8 cores = (batch n in 4) x (d-halves in 2). Zero communication.

v2: folds the h-taps into the matmul contraction (the baseline folded only
the d-taps), doubling useful MACs/cycle from 256 to 512, and stores f16
output (host converts back to f32), halving the dominant store traffic.

Stationary S[(j,hi,cc) 96, (r,ph,ho4,c') 128] per (g 32, pw 2, dwi 2):
  rows  = (d-plane j in pair, h-row hi in 6-row block, channel cc in 8)
  cols  = (d-slot r, h-parity ph, h-offset ho4 in 4, channel c')
  value = delta(cc,c') * f[kd(r,j), kh(ph, hi-ho4-1), kw(pw,dwi), g*8+c']
Each column has 2kd x 2kh = 4 nonzeros; the two w-taps (dwi) accumulate in
PSUM via w-shifted rhs windows. rhs free dim = (k-pair 2, h-block 8, w 32)
= 512 -> max-size matmuls halve the per-MM LDWEIGHTS+issue overhead.
PSUM evacuation alternates ScalarE/VectorE (ACT alone can't keep up).
"""
import sys

sys.path.insert(0, "/opt/trn_rl_repo")

from contextlib import ExitStack

import numpy as np

import concourse.bass as bass  # noqa: F401  (registers engine classes)
import concourse.tile as tile
from concourse import bacc, mybir
from concourse.bass_utils import run_bass_kernel_spmd

F32 = mybir.dt.float32
F16 = mybir.dt.float16

N_CORES = 8
TAPS = {0: [(-1, 3), (0, 1)], 1: [(0, 2), (1, 0)]}
KD = {0: (2, 0), 1: (3, 1)}  # KD[r][j]
NK = 17  # plane-pair tiles; pair index t in 0..16 holds planes (t, t+1)
NP = 9  # tile-pairs: pair p covers tiles (2p, 2p+1); tile 17 is zero pad

_PROG = None


def _build_program():
    nc = bacc.Bacc(
        "TRN2", target_bir_lowering=False, debug=False, num_devices=N_CORES
    )
    # xt: [pair, gh, (j,hi,cc), k2, gl, blk, w]
    xt_d = nc.declare_dram_parameter("xt", [NP, 2, 96, 2, 16, 8, 34], F16, isOutput=False)
    # wtab: [(j,hi,cc), m=(g,pw,dwi), (r,ph,ho4,c')]
    wt_d = nc.declare_dram_parameter("wtab", [96, 128, 128], F16, isOutput=False)
    # y: [pair, gh, q=(r,ph,ho4,c'), gl, pw, k2, blk, b]
    y_d = nc.declare_dram_parameter(
        "y", [NP, 2, 128, 16, 2, 2, 8, 32], F16, isOutput=True
    )

    with ExitStack() as ctx:
        tc = ctx.enter_context(tile.TileContext(nc))
        wpool = ctx.enter_context(tc.tile_pool(name="wpool", bufs=1))
        xpool = ctx.enter_context(tc.tile_pool(name="xpool", bufs=3))
        spool = ctx.enter_context(tc.tile_pool(name="spool", bufs=2))
        ppool = ctx.enter_context(tc.tile_pool(name="ppool", bufs=4, space="PSUM"))

        wt = wpool.tile([96, 128, 128], F16)
        wt_loaded = set()

        def load_wchunk(q):
            # lazy per-(g octet) weight chunk
            if q not in wt_loaded:
                nc.sync.dma_start(
                    out=wt[:, q * 32 : (q + 1) * 32, :],
                    in_=wt_d[:, q * 32 : (q + 1) * 32, :],
                )
                wt_loaded.add(q)

        evac_i = 0
        for gh in range(2):
            for p in range(NP):
                xt = xpool.tile([96, 2, 16, 8, 34], F16, tag="xt")
                nc.sync.dma_start(out=xt[:], in_=xt_d[p, gh])
                slab = spool.tile([128, 16, 2, 2, 8, 32], F16, tag="slab")
                nk2 = 1 if p == NP - 1 else 2  # last pair: only tile 16 valid
                for gl in range(16):
                    g = gh * 16 + gl
                    load_wchunk(g // 8)
                    ps = ppool.tile([128, 2, 2, 8, 32], F32, tag="ps", name="ps")
                    for pw in range(2):
                        for dwi in range(2):
                            m = (g * 2 + pw) * 2 + dwi
                            dw = TAPS[pw][dwi][0]
                            nc.tensor.matmul(
                                ps[:, pw, 0:nk2],
                                wt[:, m, :],
                                xt[:, 0:nk2, gl, :, 1 + dw : 33 + dw],
                                start=(dwi == 0),
                                stop=(dwi == 1),
                            )
                    # evacuate both pw banks in one op, f32 -> f16
                    eng = nc.vector if evac_i % 3 else nc.scalar
                    if eng is nc.vector:
                        nc.vector.tensor_copy(slab[:, gl], ps[:])
                    else:
                        nc.scalar.copy(slab[:, gl], ps[:])
                    evac_i += 1
                # SWDGE store keeps the Sync FIFO free for loads
                nc.gpsimd.dma_start(out=y_d[p, gh], in_=slab[:])
    nc.compile()
    return nc


def _get_program():
    global _PROG
    if _PROG is None:
        _PROG = _build_program()
    return _PROG


def _make_wtab(filters):
    ftap = np.asarray(filters, np.float32)[:, :, :, 0, :]  # (kd, kh, kw, c)
    wtab = np.zeros((96, 128, 128), np.float16)
    idx = np.arange(8)
    for g in range(32):
        for pw in range(2):
            for dwi in range(2):
                m = (g * 2 + pw) * 2 + dwi
                kw = TAPS[pw][dwi][1]
                for r in range(2):
                    for j in range(2):
                        kd = KD[r][j]
                        for ph in range(2):
                            for dh, kh in TAPS[ph]:
                                for ho4 in range(4):
                                    hi = ho4 + dh + 1
                                    wtab[
                                        j * 48 + hi * 8 + idx,
                                        m,
                                        r * 64 + ph * 32 + ho4 * 8 + idx,
                                    ] = ftap[kd, kh, kw, g * 8 + idx]
    return wtab


def _make_in_maps(x, filters):
    from numpy.lib.stride_tricks import sliding_window_view

    x = np.asarray(x, np.float32)
    wtab = _make_wtab(filters)

    in_maps = []
    for core in range(N_CORES):
        n, hf = core // 2, core % 2
        lo = 16 * hf - 1
        planes = np.zeros((18, 32, 32, 256), np.float32)
        s0, s1 = max(lo, 0), min(16 * hf + 17, 32)
        planes[s0 - lo : s1 - lo] = x[n, s0:s1]
        planes = planes.transpose(0, 3, 1, 2)  # (18, c, h, w)
        padded = np.zeros((18, 256, 34, 34), np.float16)
        padded[:, :, 1:33, 1:33] = planes  # pad index = coord + 1
        pg = padded.reshape(18, 32, 8, 34, 34)  # (plane, g, cc, H, w)
        # h-blocks: H = blk*4 + hi, hi in 0..5 -> overlapping 6-row windows
        sw = sliding_window_view(pg, 6, axis=3)[:, :, :, ::4]  # (18,32,8,8,34,6)
        xt_full = np.empty((NK, 96, 32, 8, 34), np.float16)
        for j in range(2):
            a = sw[j : j + NK]  # (17, g, cc, blk, w, hi)
            a = a.transpose(0, 5, 2, 1, 3, 4)  # (17, hi, cc, g, blk, w)
            xt_full[:, j * 48 : (j + 1) * 48] = a.reshape(NK, 48, 32, 8, 34)
        xt18 = np.concatenate(
            [xt_full, np.zeros((1, 96, 32, 8, 34), np.float16)], 0
        ).reshape(NP, 2, 96, 2, 16, 8, 34)  # (pair, k2, p, gh, gl, blk, w)
        xt2 = np.ascontiguousarray(xt18.transpose(0, 3, 2, 1, 4, 5, 6))
        in_maps.append({"xt": xt2, "wtab": wtab})
    return in_maps


def kernel(x, filters):
    nc = _get_program()
    in_maps = _make_in_maps(x, filters)
    res = run_bass_kernel_spmd(nc, in_maps, list(range(N_CORES)))
    y = np.empty((4, 64, 64, 64, 256), np.float32)
    for core in range(N_CORES):
        n, hf = core // 2, core % 2
        yc = res.results[core]["y"]  # [pair, gh, q, gl, pw, k2, blk, b] f16
        yc = yc.reshape(NP, 2, 2, 2, 4, 8, 16, 2, 2, 8, 32)
        # dims: pair0 gh1 r2 ph3 ho4_4 cp5 gl6 pw7 k2_8 blk9 b10
        yt = yc.transpose(0, 8, 2, 9, 4, 3, 10, 7, 1, 6, 5)
        # (pair, k2, r, blk, ho4, ph, b, pw, gh, gl, cp)
        yt = yt.reshape(36, 64, 64, 256)[1:33]
        y[n, 32 * hf : 32 * hf + 32] = yt.astype(np.float32)
    return y


# revision 3
# speedup vs baseline: 1.1983x; 1.1197x over previous
"""Depthwise 3D transposed conv (stride 2, k=4, SAME) on 8 trn2 NeuronCores.

x: (4, 32, 32, 32, 256) f32, filters: (4, 4, 4, 1, 256) f32
y: (4, 64, 64, 64, 256) f32

Sharding: 8 cores = (batch n in 4) x (d-halves 2). Zero communication.

Compute structure (v2): h-taps folded into the matmul contraction.
Stationary S[(j,hi,cc) 96, (r,ph,ho4,c') 128] per (g 32, pw 2, dwi 2)
has 4 nonzeros/column (2 kd x 2 kh) = 512 useful MACs/cycle; the two
w-taps (dwi) accumulate in PSUM via w-shifted rhs windows; rhs free =
(k2 2, blk 8, b 32) = 512 (two plane-pair tiles per matmul). f16 stores.

v4 scheduling fixes (from the v3 trace):
 - plane-dedup SBUF->SBUF copies ride the Sync HWDGE ring, not the
   Scalar ring (they were queuing behind 1.1us ACTIVATE evacuations,
   starving the PE ~5us/pair)
 - half-pair slabs with bufs=5 break the store->evac->PSUM->PE
   backpressure chain
 - k2-major store layout keeps every store contiguous per partition
 - pair 0 loads all three planes straight from HBM (no copy on the
   startup critical path); first weight chunk is 8 matrices, not 32
"""
import sys

sys.path.insert(0, "/opt/trn_rl_repo")

from contextlib import ExitStack

import numpy as np

import concourse.bass as bass  # noqa: F401  (registers engine classes)
import concourse.tile as tile
from concourse import bacc, mybir
from concourse.bass_utils import run_bass_kernel_spmd

F32 = mybir.dt.float32
F16 = mybir.dt.float16

N_CORES = 8
TAPS = {0: [(-1, 3), (0, 1)], 1: [(0, 2), (1, 0)]}
KD = {0: (2, 0), 1: (3, 1)}  # KD[r][j]
NK = 17  # plane-pair tiles; tile t holds planes (t, t+1)
NP = 9  # tile-pairs: pair p covers tiles (2p, 2p+1); tile 17 is dropped
WCHUNKS = [(0, 8), (8, 32), (32, 64), (64, 96), (96, 128)]

_PROG = None


def _build_program():
    nc = bacc.Bacc(
        "TRN2", target_bir_lowering=False, debug=False, num_devices=N_CORES
    )
    # xp: one copy of each plane: [gh, plane q, (hi,cc) 48, gl, blk, w]
    xp_d = nc.declare_dram_parameter("xp", [2, 18, 48, 16, 8, 34], F16, isOutput=False)
    # wtab: [(j,hi,cc), m=(g,pw,dwi), (r,ph,ho4,c')]
    wt_d = nc.declare_dram_parameter("wtab", [96, 128, 128], F16, isOutput=False)
    # y: [pair, gh, glh, q=(r,ph,ho4,c'), k2, gl, pw, blk, b]
    y_d = nc.declare_dram_parameter(
        "y", [NP, 2, 2, 128, 2, 8, 2, 8, 32], F16, isOutput=True
    )

    with ExitStack() as ctx:
        tc = ctx.enter_context(tile.TileContext(nc))
        wpool = ctx.enter_context(tc.tile_pool(name="wpool", bufs=1))
        xpool = ctx.enter_context(tc.tile_pool(name="xpool", bufs=4))
        spool = ctx.enter_context(tc.tile_pool(name="spool", bufs=5))
        ppool = ctx.enter_context(tc.tile_pool(name="ppool", bufs=4, space="PSUM"))

        wt = wpool.tile([96, 128, 128], F16)
        wt_loaded = set()

        def load_wchunk(g):
            for ci, (m0, m1) in enumerate(WCHUNKS):
                if m0 <= g * 4 < m1 and ci not in wt_loaded:
                    nc.sync.dma_start(
                        out=wt[:, m0:m1, :], in_=wt_d[:, m0:m1, :]
                    )
                    wt_loaded.add(ci)

        load_wchunk(0)  # before any tile loads: overlaps the first big DMA
        evac_i = 0
        for gh in range(2):
            prev = None
            for p in range(NP):
                xt = xpool.tile([96, 2, 16, 8, 34], F16, tag="xt")
                nk2 = 1 if p == NP - 1 else 2
                if p < NP - 1:
                    # planes (2p+1, 2p+2) -> (j0,k2=1), (j1,k2=1): one DMA
                    nc.sync.dma_start(
                        out=xt[:, 1],
                        in_=xp_d[gh, 2 * p + 1 : 2 * p + 3].rearrange(
                            "p a gl blk w -> (p a) gl blk w"
                        ),
                    )
                else:
                    # last pair: only tile 16 (k2=0); plane 17 -> (j1,k2=0)
                    nc.sync.dma_start(out=xt[48:96, 0], in_=xp_d[gh, 17])
                if p == 0:
                    # both k2=0 planes straight from HBM: nothing on the
                    # startup critical path but independent loads
                    nc.sync.dma_start(
                        out=xt[:, 0],
                        in_=xp_d[gh, 0:2].rearrange(
                            "p a gl blk w -> (p a) gl blk w"
                        ),
                    )
                else:
                    # plane 2p -> (j0,k2=0): copy from previous pair tile
                    nc.sync.dma_start(out=xt[0:48, 0], in_=prev[48:96, 1])
                    if p < NP - 1:
                        # plane 2p+1 -> (j1,k2=0): same-tile copy
                        nc.sync.dma_start(out=xt[48:96, 0], in_=xt[0:48, 1])
                prev = xt

                for glh in range(2):
                    slab = spool.tile([128, 2, 8, 2, 8, 32], F16, tag="slab")
                    for gl8 in range(8):
                        gl = glh * 8 + gl8
                        g = gh * 16 + gl
                        load_wchunk(g)
                        ps = ppool.tile([128, 2, 2, 8, 32], F32, tag="ps", name="ps")
                        for pw in range(2):
                            for dwi in range(2):
                                m = (g * 2 + pw) * 2 + dwi
                                dw = TAPS[pw][dwi][0]
                                nc.tensor.matmul(
                                    ps[:, pw, 0:nk2],
                                    wt[:, m, :],
                                    xt[:, 0:nk2, gl, :, 1 + dw : 33 + dw],
                                    start=(dwi == 0),
                                    stop=(dwi == 1),
                                )
                        out_ap = slab[:, :, gl8].rearrange(
                            "q k2 pw blk b -> q pw k2 blk b"
                        )
                        if evac_i % 2:
                            nc.vector.tensor_copy(out_ap, ps[:])
                        else:
                            nc.scalar.copy(out_ap, ps[:])
                        evac_i += 1
                    # SWDGE stores; skip out-of-range boundary slots (p=0:
                    # r=0 of tile 0 is plane -1; p=8: only tile 16 r=0)
                    if p == 0:
                        nc.gpsimd.dma_start(
                            out=y_d[0, gh, glh, 64:128], in_=slab[64:128]
                        )
                        nc.gpsimd.dma_start(
                            out=y_d[0, gh, glh, 0:64, 1], in_=slab[0:64, 1]
                        )
                    elif p == NP - 1:
                        nc.gpsimd.dma_start(
                            out=y_d[p, gh, glh, 0:64, 0], in_=slab[0:64, 0]
                        )
                    else:
                        nc.gpsimd.dma_start(out=y_d[p, gh, glh], in_=slab[:])
    nc.compile()
    return nc


def _get_program():
    global _PROG
    if _PROG is None:
        _PROG = _build_program()
    return _PROG


def _make_wtab(filters):
    ftap = np.asarray(filters, np.float32)[:, :, :, 0, :]  # (kd, kh, kw, c)
    wtab = np.zeros((96, 128, 128), np.float16)
    idx = np.arange(8)
    for g in range(32):
        for pw in range(2):
            for dwi in range(2):
                m = (g * 2 + pw) * 2 + dwi
                kw = TAPS[pw][dwi][1]
                for r in range(2):
                    for j in range(2):
                        kd = KD[r][j]
                        for ph in range(2):
                            for dh, kh in TAPS[ph]:
                                for ho4 in range(4):
                                    hi = ho4 + dh + 1
                                    wtab[
                                        j * 48 + hi * 8 + idx,
                                        m,
                                        r * 64 + ph * 32 + ho4 * 8 + idx,
                                    ] = ftap[kd, kh, kw, g * 8 + idx]
    return wtab


def _make_in_maps(x, filters):
    from numpy.lib.stride_tricks import sliding_window_view

    x = np.asarray(x, np.float32)
    wtab = _make_wtab(filters)

    in_maps = []
    for core in range(N_CORES):
        n, hf = core // 2, core % 2
        lo = 16 * hf - 1
        planes = np.zeros((18, 32, 32, 256), np.float32)
        s0, s1 = max(lo, 0), min(16 * hf + 17, 32)
        planes[s0 - lo : s1 - lo] = x[n, s0:s1]
        planes = planes.transpose(0, 3, 1, 2)  # (18, c, h, w)
        padded = np.zeros((18, 256, 34, 34), np.float16)
        padded[:, :, 1:33, 1:33] = planes  # pad index = coord + 1
        pg = padded.reshape(18, 32, 8, 34, 34)  # (plane, g, cc, H, w)
        # h-blocks: H = blk*4 + hi, hi in 0..5 -> overlapping 6-row windows
        sw = sliding_window_view(pg, 6, axis=3)[:, :, :, ::4]  # (18,32,8,8,34,6)
        a = sw.transpose(0, 5, 2, 1, 3, 4)  # (plane, hi, cc, g, blk, w)
        xp = a.reshape(18, 48, 2, 16, 8, 34).transpose(2, 0, 1, 3, 4, 5)
        in_maps.append({"xp": np.ascontiguousarray(xp), "wtab": wtab})
    return in_maps


def kernel(x, filters):
    nc = _get_program()
    in_maps = _make_in_maps(x, filters)
    res = run_bass_kernel_spmd(nc, in_maps, list(range(N_CORES)))
    y = np.empty((4, 64, 64, 64, 256), np.float32)
    for core in range(N_CORES):
        n, hf = core // 2, core % 2
        yc = res.results[core]["y"]  # [pair, gh, glh, q, k2, gl8, pw, blk, b]
        yc = yc.reshape(NP, 2, 2, 2, 2, 4, 8, 2, 8, 2, 8, 32)
        # dims: pair0 gh1 glh2 r3 ph4 ho4_5 cp6 k2_7 gl8_8 pw9 blk10 b11
        yt = yc.transpose(0, 7, 3, 10, 5, 4, 11, 9, 1, 2, 8, 6)
        # (pair, k2, r, blk, ho4, ph, b, pw, gh, glh, gl8, cp)
        yt = yt.reshape(36, 64, 64, 256)[1:33]
        y[n, 32 * hf : 32 * hf + 32] = yt.astype(np.float32)
    return y


# revision 4
# speedup vs baseline: 1.2063x; 1.0067x over previous
"""Depthwise 3D transposed conv (stride 2, k=4, SAME) on 8 trn2 NeuronCores.

x: (4, 32, 32, 32, 256) f32, filters: (4, 4, 4, 1, 256) f32
y: (4, 64, 64, 64, 256) f32

Sharding: 8 cores = (batch n in 4) x (d-halves 2). Zero communication.

Compute structure (v2): h-taps folded into the matmul contraction.
Stationary S[(j,hi,cc) 96, (r,ph,ho4,c') 128] per (g 32, pw 2, dwi 2)
has 4 nonzeros/column (2 kd x 2 kh) = 512 useful MACs/cycle; the two
w-taps (dwi) accumulate in PSUM via w-shifted rhs windows; rhs free =
(k2 2, blk 8, b 32) = 512 (two plane-pair tiles per matmul). f16 stores.

v4 scheduling fixes (from the v3 trace):
 - duplicated planes are re-read from HBM at full DMA width (the
   48-partition SBUF->SBUF dedup copies were slower than HBM re-reads
   and stretched matmul pacing via SBUF port contention)
 - half-pair slabs with bufs=5 break the store->evac->PSUM->PE
   backpressure chain
 - k2-major store layout keeps every store contiguous per partition
 - pair 0 loads all three planes straight from HBM (no copy on the
   startup critical path); first weight chunk is 8 matrices, not 32
"""
import sys

sys.path.insert(0, "/opt/trn_rl_repo")

from contextlib import ExitStack

import numpy as np

import concourse.bass as bass  # noqa: F401  (registers engine classes)
import concourse.tile as tile
from concourse import bacc, mybir
from concourse.bass_utils import run_bass_kernel_spmd

F32 = mybir.dt.float32
F16 = mybir.dt.float16

N_CORES = 8
TAPS = {0: [(-1, 3), (0, 1)], 1: [(0, 2), (1, 0)]}
KD = {0: (2, 0), 1: (3, 1)}  # KD[r][j]
NK = 17  # plane-pair tiles; tile t holds planes (t, t+1)
NP = 9  # tile-pairs: pair p covers tiles (2p, 2p+1); tile 17 is dropped
WCHUNKS = [(0, 8), (8, 32), (32, 64), (64, 96), (96, 128)]

_PROG = None


def _build_program():
    nc = bacc.Bacc(
        "TRN2", target_bir_lowering=False, debug=False, num_devices=N_CORES
    )
    # xp: one copy of each plane: [gh, plane q, (hi,cc) 48, gl, blk, w]
    xp_d = nc.declare_dram_parameter("xp", [2, 18, 48, 16, 8, 34], F16, isOutput=False)
    # wtab: [(j,hi,cc), m=(g,pw,dwi), (r,ph,ho4,c')]
    wt_d = nc.declare_dram_parameter("wtab", [96, 128, 128], F16, isOutput=False)
    # y: [pair, gh, glh, q=(r,ph,ho4,c'), k2, gl, pw, blk, b]
    y_d = nc.declare_dram_parameter(
        "y", [NP, 2, 2, 128, 2, 8, 2, 8, 32], F16, isOutput=True
    )

    with ExitStack() as ctx:
        tc = ctx.enter_context(tile.TileContext(nc))
        wpool = ctx.enter_context(tc.tile_pool(name="wpool", bufs=1))
        xpool = ctx.enter_context(tc.tile_pool(name="xpool", bufs=4))
        spool = ctx.enter_context(tc.tile_pool(name="spool", bufs=5))
        ppool = ctx.enter_context(tc.tile_pool(name="ppool", bufs=4, space="PSUM"))

        wt = wpool.tile([96, 128, 128], F16)
        wt_loaded = set()

        def load_wchunk(g):
            for ci, (m0, m1) in enumerate(WCHUNKS):
                if m0 <= g * 4 < m1 and ci not in wt_loaded:
                    nc.sync.dma_start(
                        out=wt[:, m0:m1, :], in_=wt_d[:, m0:m1, :]
                    )
                    wt_loaded.add(ci)

        load_wchunk(0)  # before any tile loads: overlaps the first big DMA
        evac_i = 0
        for gh in range(2):
            prev = None
            for p in range(NP):
                xt = xpool.tile([96, 2, 16, 8, 34], F16, tag="xt")
                nk2 = 1 if p == NP - 1 else 2
                # full-width plane-pair loads; duplicated planes re-read
                # from HBM (48-partition SBUF copies measured ~2x slower
                # per byte and stretched matmul pacing via port contention)
                for k2 in range(nk2):
                    src_ap = xp_d[gh, 2 * p + k2 : 2 * p + k2 + 2].rearrange(
                        "p a gl blk w -> (p a) gl blk w"
                    )
                    if p == 0 and gh == 0:
                        # split first loads so matmuls start on gl 0-7
                        # while gl 8-15 is still in flight
                        for glh2 in range(2):
                            sl = slice(glh2 * 8, glh2 * 8 + 8)
                            nc.sync.dma_start(
                                out=xt[:, k2, sl], in_=src_ap[:, sl]
                            )
                    else:
                        nc.sync.dma_start(out=xt[:, k2], in_=src_ap)

                for glh in range(2):
                    slab = spool.tile([128, 2, 8, 2, 8, 32], F16, tag="slab")
                    for gl8 in range(8):
                        gl = glh * 8 + gl8
                        g = gh * 16 + gl
                        load_wchunk(g)
                        ps = ppool.tile([128, 2, 2, 8, 32], F32, tag="ps", name="ps")
                        for pw in range(2):
                            for dwi in range(2):
                                m = (g * 2 + pw) * 2 + dwi
                                dw = TAPS[pw][dwi][0]
                                nc.tensor.matmul(
                                    ps[:, pw, 0:nk2],
                                    wt[:, m, :],
                                    xt[:, 0:nk2, gl, :, 1 + dw : 33 + dw],
                                    start=(dwi == 0),
                                    stop=(dwi == 1),
                                )
                        out_ap = slab[:, :, gl8].rearrange(
                            "q k2 pw blk b -> q pw k2 blk b"
                        )
                        if evac_i % 2:
                            nc.vector.tensor_copy(out_ap, ps[:])
                        else:
                            nc.scalar.copy(out_ap, ps[:])
                        evac_i += 1
                    # SWDGE stores; skip out-of-range boundary slots (p=0:
                    # r=0 of tile 0 is plane -1; p=8: only tile 16 r=0)
                    if p == 0:
                        nc.gpsimd.dma_start(
                            out=y_d[0, gh, glh, 64:128], in_=slab[64:128]
                        )
                        nc.gpsimd.dma_start(
                            out=y_d[0, gh, glh, 0:64, 1], in_=slab[0:64, 1]
                        )
                    elif p == NP - 1:
                        nc.gpsimd.dma_start(
                            out=y_d[p, gh, glh, 0:64, 0], in_=slab[0:64, 0]
                        )
                    else:
                        nc.gpsimd.dma_start(out=y_d[p, gh, glh], in_=slab[:])
    nc.compile()
    return nc


def _get_program():
    global _PROG
    if _PROG is None:
        _PROG = _build_program()
    return _PROG


def _make_wtab(filters):
    ftap = np.asarray(filters, np.float32)[:, :, :, 0, :]  # (kd, kh, kw, c)
    wtab = np.zeros((96, 128, 128), np.float16)
    idx = np.arange(8)
    for g in range(32):
        for pw in range(2):
            for dwi in range(2):
                m = (g * 2 + pw) * 2 + dwi
                kw = TAPS[pw][dwi][1]
                for r in range(2):
                    for j in range(2):
                        kd = KD[r][j]
                        for ph in range(2):
                            for dh, kh in TAPS[ph]:
                                for ho4 in range(4):
                                    hi = ho4 + dh + 1
                                    wtab[
                                        j * 48 + hi * 8 + idx,
                                        m,
                                        r * 64 + ph * 32 + ho4 * 8 + idx,
                                    ] = ftap[kd, kh, kw, g * 8 + idx]
    return wtab


def _make_in_maps(x, filters):
    from numpy.lib.stride_tricks import sliding_window_view

    x = np.asarray(x, np.float32)
    wtab = _make_wtab(filters)

    in_maps = []
    for core in range(N_CORES):
        n, hf = core // 2, core % 2
        lo = 16 * hf - 1
        planes = np.zeros((18, 32, 32, 256), np.float32)
        s0, s1 = max(lo, 0), min(16 * hf + 17, 32)
        planes[s0 - lo : s1 - lo] = x[n, s0:s1]
        planes = planes.transpose(0, 3, 1, 2)  # (18, c, h, w)
        padded = np.zeros((18, 256, 34, 34), np.float16)
        padded[:, :, 1:33, 1:33] = planes  # pad index = coord + 1
        pg = padded.reshape(18, 32, 8, 34, 34)  # (plane, g, cc, H, w)
        # h-blocks: H = blk*4 + hi, hi in 0..5 -> overlapping 6-row windows
        sw = sliding_window_view(pg, 6, axis=3)[:, :, :, ::4]  # (18,32,8,8,34,6)
        a = sw.transpose(0, 5, 2, 1, 3, 4)  # (plane, hi, cc, g, blk, w)
        xp = a.reshape(18, 48, 2, 16, 8, 34).transpose(2, 0, 1, 3, 4, 5)
        in_maps.append({"xp": np.ascontiguousarray(xp), "wtab": wtab})
    return in_maps


def kernel(x, filters):
    nc = _get_program()
    in_maps = _make_in_maps(x, filters)
    res = run_bass_kernel_spmd(nc, in_maps, list(range(N_CORES)))
    y = np.empty((4, 64, 64, 64, 256), np.float32)
    for core in range(N_CORES):
        n, hf = core // 2, core % 2
        yc = res.results[core]["y"]  # [pair, gh, glh, q, k2, gl8, pw, blk, b]
        yc = yc.reshape(NP, 2, 2, 2, 2, 4, 8, 2, 8, 2, 8, 32)
        # dims: pair0 gh1 glh2 r3 ph4 ho4_5 cp6 k2_7 gl8_8 pw9 blk10 b11
        yt = yc.transpose(0, 7, 3, 10, 5, 4, 11, 9, 1, 2, 8, 6)
        # (pair, k2, r, blk, ho4, ph, b, pw, gh, glh, gl8, cp)
        yt = yt.reshape(36, 64, 64, 256)[1:33]
        y[n, 32 * hf : 32 * hf + 32] = yt.astype(np.float32)
    return y
